# revision 32
# baseline (speedup 1.0000x reference)
"""AutoCorrelation (Autoformer-style) sparse attention kernel for 8 trn2 cores.

Math (exact refactoring of the reference):
  mean_corr[b,j] = <(sum_i q[b,i]) @ (wq@wk.T), keys[b,j]> / (H*L)
  top7 delays d_k + softmax weights w_k over mean_corr
  out[b,l]      = sum_k w_k * values[b,(l+d_k)%L] @ (wv@wo)

Sharding: core c handles batch b=c//2, output half h=c%2.

v6 schedule (cost-model driven, verified on HW):
  - host-folded weights m = wq@wk.T, w2 = wv@wo; host-extended
    vte = [vrot.T | vrot.T[:, :HALF]] (no on-device wrap copy)
  - 3 DMA queues; a dma_start blocks its issuing engine until the
    transfer ends, so ACT only carries DMAs that finish before its first
    compute, Pool's SWDGE desc-gens run while Pool is otherwise idle,
    and all late bulk rides SP
  - qsum: PE matmuls over early q DMAs + DVE pair-adds for the late
    halves; single transpose hop (tiny [1,128]x[1,1] matmuls into one
    [128,4] psum tile); uT computed directly as 16 single-column
    matmuls (no second hop); s-matmuls lc-major over 5 skewed psum
    banks [512,512,512,384,128] so banks retire in a stream
  - topk without MaxIndex: scores quantized to integers on ACT via the
    f32 +1.5*2^23 trick, packed p = t*4096 + col_idx on Pool (pure
    arithmetic - gpsimd cannot touch PSUM or bitwise ops), per-bank
    top8 via DVE Max, one merge Max; idx/vals recovered arithmetically
  - softmax: max-subtraction makes w0 == 1 so all k0 mix terms are
    plain copies; 1/sum(exp) is broadcast separately into rse_bc and
    applied by the final psum->sbuf copies
  - mix: PE 4 chunks (scaled-identity matmuls) + all 8 finals; DVE 2
    chunks (stt); ACT+Pool 2 chunks (ACT dyn-muls, Pool adds); final
    psums alternate decoupled bank tags; out DMA on SP
"""

import numpy as np
from contextlib import ExitStack

import concourse.bass as bass
import concourse.bacc as bacc
import concourse.mybir as mybir
import concourse.tile as tile
from concourse.bass_utils import run_bass_kernel_spmd

B, L, D, H = 4, 2048, 512, 8
HALF = L // 2
KTOP = 7
EXT = L + HALF
P = 128
FT = D // P
NT = L // P
NCH = HALF // P
F32 = mybir.dt.float32
BF16 = mybir.dt.bfloat16
U32 = mybir.dt.uint32
I32 = mybir.dt.int32
AF = mybir.ActivationFunctionType
ALU = mybir.AluOpType
ENG = mybir.EngineType

QBIAS = 12582912.0  # 1.5*2^23: +QBIAS rounds f32 to integer, ulp stays 1

N_DUMMY = 7        # PE warm-keepers through the topk window
DUMMY_FLOOR = 0.0122  # ms floor for the warm-keepers


def _build():
    nc = bacc.Bacc()
    q_d = nc.dram_tensor("q", [L, D], BF16, kind="ExternalInput")
    kt_d = nc.dram_tensor("kt", [D, L], BF16, kind="ExternalInput")
    vte_d = nc.dram_tensor("vte", [D, EXT], BF16, kind="ExternalInput")
    m_d = nc.dram_tensor("m", [D, D], BF16, kind="ExternalInput")
    w2_d = nc.dram_tensor("w2", [D, D], BF16, kind="ExternalInput")
    cst_d = nc.dram_tensor("cst", [P, 257], BF16, kind="ExternalInput")
    out_d = nc.dram_tensor("out", [HALF, D], BF16, kind="ExternalOutput")

    qdr = q_d.rearrange("(t p) c -> p t c", p=P)
    ktdr = kt_d.rearrange("(f p) l -> p f l", p=P)
    vtdr = vte_d.rearrange("(f p) l -> p f l", p=P)
    mdr = m_d.rearrange("(f p) c -> p f c", p=P)
    w2dr = w2_d.rearrange("(f p) c -> p f c", p=P)

    with tile.TileContext(nc) as tc, ExitStack() as ctx:
        big = ctx.enter_context(tc.tile_pool(name="big", bufs=1))
        sm = ctx.enter_context(tc.tile_pool(name="sm", bufs=1))
        psp = ctx.enter_context(
            tc.tile_pool(name="psp", bufs=1, space=bass.MemorySpace.PSUM)
        )

        cst = sm.tile([P, 257], BF16, tag="cst")
        qp = big.tile([P, NT, D], BF16, tag="qp")
        qh = big.tile([P, 2, D], BF16, tag="qh")
        ktp = big.tile([P, FT, L], BF16, tag="ktp")
        vt_sb = big.tile([P, FT, EXT], BF16, tag="vt")
        mp = big.tile([P, FT, D], BF16, tag="mp")
        w2p = big.tile([P, FT, D], BF16, tag="w2p")
        warm = sm.tile([P, P], BF16, tag="warm")

        # ---- DMA plan (3 queues: SP / ACT / Pool-SWDGE) ------------------
        # SP  : cst | q(0,1) q(2,3) q(4,5) | m01 | kt1 | vte f0 | out x3
        # ACT : q(6,7) q(8,9) | m23 | w2 | vte f1 | vte f3 | out x3
        # Pool: q(12,13) q(14,15) q(10,11) | kt2 kt3 kt0 | vte f2 | out x2
        nc.sync.dma_start(cst[:], cst_d[:])
        ident = cst[:, 0:128]
        ones_col = cst[:, 128:129]
        one_one = cst[0:1, 128:129]
        ones_row = cst[0:1, 129:257]

        nc.sync.dma_start(qp[:, 0:2, :], qdr[:, 0:2, :])
        nc.scalar.dma_start(qp[:, 6:8, :], qdr[:, 6:8, :])
        nc.gpsimd.dma_start(qp[:, 12:14, :], qdr[:, 12:14, :])
        nc.sync.dma_start(qp[:, 2:4, :], qdr[:, 2:4, :])
        nc.scalar.dma_start(qp[:, 8:10, :], qdr[:, 8:10, :])
        nc.gpsimd.dma_start(qp[:, 14:16, :], qdr[:, 14:16, :])
        nc.sync.dma_start(qp[:, 4:6, :], qdr[:, 4:6, :])
        nc.gpsimd.dma_start(qp[:, 10:12, :], qdr[:, 10:12, :])

        nc.sync.dma_start(mp[:, 0:2, :], mdr[:, 0:2, :])
        nc.scalar.dma_start(mp[:, 2:4, :], mdr[:, 2:4, :])
        nc.gpsimd.dma_start(ktp[:, 2:3, :], ktdr[:, 2:3, :])
        nc.sync.dma_start(ktp[:, 1:2, :], ktdr[:, 1:2, :])
        nc.gpsimd.dma_start(ktp[:, 3:4, :], ktdr[:, 3:4, :])
        nc.scalar.dma_start(w2p[:], w2dr[:])
        nc.gpsimd.dma_start(ktp[:, 0:1, 0:HALF], ktdr[:, 0:1, 0:HALF])
        nc.gpsimd.dma_start(ktp[:, 0:1, HALF:L], ktdr[:, 0:1, HALF:L])
        nc.sync.dma_start(vt_sb[:, 0:1, :], vtdr[:, 0:1, :])
        nc.scalar.dma_start(vt_sb[:, 2:3, :], vtdr[:, 2:3, :])

        # index row (exact small integers in f32) on Pool between desc-gens
        civ = sm.tile([1, L], F32, tag="civ")
        nc.gpsimd.iota(
            civ[0:1, 0:L], [[1, L]], base=0, channel_multiplier=0,
            allow_small_or_imprecise_dtypes=True,
        )

        # ---- small tiles -------------------------------------------------
        aux2 = sm.tile([P, 32], BF16, tag="aux2")
        qsT = aux2[:, 0:4]
        uT = aux2[:, 4:8]
        exbf = aux2[0:1, 16:24]
        srow = sm.tile([1, 2 * D], BF16, tag="srow")
        qsum_sb = srow[0:1, 0:D]
        u_sb = srow[0:1, D : 2 * D]
        sfl = sm.tile([1, L], F32, tag="sfl")
        stmp = sm.tile([1, 2 * D], F32, tag="stmp")
        aux3 = sm.tile([1, 64], F32, tag="aux3")
        aux4 = sm.tile([1, 40], F32, tag="aux4")
        vq = aux4[0:1, 0:40]
        vals8 = aux3[0:1, 40:48]
        a8 = aux3[0:1, 48:56]
        t8 = aux3[0:1, 8:16]
        i8f = aux3[0:1, 16:24]
        exf = aux3[0:1, 56:63]
        exin = aux3[0:1, 24:31]
        se = aux3[0:1, 63:64]
        wbc = sm.tile([P, 8], F32, tag="wbc")
        rse = sm.tile([1, 1], F32, tag="rse")
        rse_bc = sm.tile([P, 1], F32, tag="rsebc")
        onesf = sm.tile([1, P], F32, tag="onesf")
        onesf_row = onesf[0:1, :]
        idx8 = sm.tile([1, 8], U32, tag="idx8")
        sid = sm.tile([P, KTOP, P], BF16, tag="sid")
        mixs = big.tile([P, NCH, FT, P], BF16, tag="mixs")
        atmp = sm.tile([P, 8, FT, P], BF16, tag="atmp")
        ostg = sm.tile([P, 4, D], BF16, tag="ostg")

        # ---- PE warm-up: get the p-state ramp going before qsum ----------
        ps_dum = psp.tile([P, D], F32, tag="b", bufs=2)
        nc.vector.memset(warm[:], 0.0)
        nc.vector.memset(onesf[:], 1.0)
        for _ in range(4):
            nc.tensor.matmul(
                ps_dum[:, 0:P], warm[:], warm[:], start=True, stop=True,
                skip_group_check=True,
            )

        # ---- qsum: raw matmuls on early q DMAs + DVE pair-adds on late ---
        nc.vector.tensor_tensor(qh[:, 0, :], qp[:, 8, :], qp[:, 9, :], ALU.add)
        nc.vector.tensor_tensor(qh[:, 1, :], qp[:, 4, :], qp[:, 5, :], ALU.add)

        ps_qsum = psp.tile([1, D], F32, tag="a", bufs=2)
        raw = [12, 13, 0, 1, 14, 15, 6, 7, 2, 3, 10, 11]
        for i, t in enumerate(raw):
            nc.tensor.matmul(
                ps_qsum[:], ones_col, qp[:, t, :], start=(i == 0), stop=False
            )
        for i in range(2):
            nc.tensor.matmul(
                ps_qsum[:], ones_col, qh[:, i, :], start=False, stop=(i == 1)
            )

        def transpose_hop(ps_row, row_sb, colT, hid):
            # half-row copies on DVE, 4 tiny PE transposes into ONE
            # [128,4] psum tile, single col copy out
            nc.vector.tensor_copy(row_sb[0:1, 0 : 2 * P], ps_row[0:1, 0 : 2 * P])
            nc.vector.tensor_copy(row_sb[0:1, 2 * P : 4 * P], ps_row[0:1, 2 * P : 4 * P])
            psT = psp.tile([P, 4], F32, tag="a", bufs=2, name=f"psT{hid}")
            for c in range(FT):
                lo, hi = c * P, (c + 1) * P
                nc.tensor.matmul(
                    psT[:, c : c + 1], row_sb[0:1, lo:hi], one_one,
                    start=True, stop=True, skip_group_check=True,
                )
            nc.vector.tensor_copy(colT[:], psT[:])

        transpose_hop(ps_qsum, qsum_sb, qsT, 0)
        # uT = (qsum @ m)^T computed directly: per output column block,
        # accumulate 4 single-column matmuls -> no second transpose hop
        ps_uT = psp.tile([P, 4], F32, tag="a", bufs=2)
        for dpb in range(FT):
            for i in range(FT):
                nc.tensor.matmul(
                    ps_uT[:, dpb : dpb + 1],
                    mp[:, i, dpb * P : (dpb + 1) * P],
                    qsT[:, i : i + 1],
                    start=(i == 0),
                    stop=(i == FT - 1),
                    skip_group_check=True,
                )
        nc.vector.tensor_copy(uT[:], ps_uT[:])

        # remaining vt pieces ride the idle SP queue
        nc.sync.dma_start(vt_sb[:, 1:2, :], vtdr[:, 1:2, :])
        nc.sync.dma_start(vt_sb[:, 3:4, :], vtdr[:, 3:4, :])

        # ---- scores: 5 skewed banks so the last retires almost with the
        # s-matmul stream end (tiny tail bank -> earlier topk) -------------
        BW = [512, 512, 512, 384, 128]
        BO = [0, 512, 1024, 1536, 1920]
        ps_s = [
            psp.tile([1, BW[i]], F32, tag="s", bufs=4, name=f"ps_s{i}")
            for i in range(5)
        ]
        # s-matmuls interleaved with the pack pipeline per bank so waits
        # attach tightly: ACT quantizes scores to integers (f32 +1.5*2^23
        # trick), Pool packs p = t*4096 + col_idx, DVE runs per-bank maxes
        ford = [2, 3, 1, 0]  # kt piece arrival order
        for lc in range(5):
            for fi, f in enumerate(ford):
                nc.tensor.matmul(
                    ps_s[lc][:],
                    uT[:, f : f + 1],
                    ktp[:, f, BO[lc] : BO[lc] + BW[lc]],
                    start=(fi == 0),
                    stop=(fi == FT - 1),
                )
            stg = stmp[0:1, (lc % 2) * D : (lc % 2) * D + BW[lc]]
            sfs = sfl[0:1, BO[lc] : BO[lc] + BW[lc]]
            nc.scalar.activation(stg, ps_s[lc][:], AF.Copy, bias=QBIAS)
            nc.gpsimd.tensor_scalar(
                sfs, stg, -QBIAS, 4096.0, ALU.add, ALU.mult
            )
            nc.gpsimd.tensor_tensor(
                sfs, sfs, civ[0:1, BO[lc] : BO[lc] + BW[lc]], ALU.add
            )
            nc.vector.max(vq[0:1, 8 * lc : 8 * lc + 8], sfs)
        nc.vector.max(vals8, vq)

        # unpack: t = round(p/4096) via the +QBIAS trick, idx = p - 4096*t
        nc.vector.tensor_scalar(
            a8, vals8, 1.0 / 4096.0, QBIAS, ALU.mult, ALU.add
        )
        nc.vector.tensor_scalar(t8, a8, -QBIAS, None, ALU.add)
        nc.vector.scalar_tensor_tensor(
            i8f, t8, -4096.0, vals8, ALU.mult, ALU.add
        )
        nc.vector.tensor_copy(idx8[0:1, :], i8f)
        nc.vector.tensor_scalar_sub(exin, t8[0:1, 0:KTOP], t8[0:1, 0:1])
        nc.scalar.activation(exf, exin, AF.Exp, scale=1.0 / (H * L))
        # broadcast UNNORMALIZED weights (w0 == 1 exactly, so the k0 mix
        # terms are plain copies); 1/sum(exp) rides a parallel path into
        # rse_bc and is applied by the final psum->sbuf copies
        nc.gpsimd.tensor_copy(exbf[0:1, 0:KTOP], exf)
        nc.vector.tensor_reduce(se, exf, mybir.AxisListType.X, ALU.add)
        ps_w = psp.tile([P, 8], F32, tag="a", bufs=2)
        nc.tensor.matmul(
            ps_w[:, 0:KTOP], ones_row, exbf[0:1, 0:KTOP], start=True, stop=True
        )
        nc.vector.tensor_copy(wbc[:, 0:KTOP], ps_w[:, 0:KTOP])
        for k in range(1, KTOP):
            nc.gpsimd.tensor_scalar_mul(sid[:, k, :], ident, wbc[:, k : k + 1])

        nc.vector.reciprocal(rse[:], se)
        ps_r = psp.tile([P, 1], F32, tag="a", bufs=2)
        nc.tensor.matmul(ps_r[:], onesf_row, rse[:], start=True, stop=True)
        nc.vector.tensor_copy(rse_bc[:], ps_r[:])

        # PE warm-keepers spanning the Max window
        with tc.tile_wait_until(DUMMY_FLOOR):
            for i in range(N_DUMMY):
                nc.tensor.matmul(
                    ps_dum[:], ident, w2p[:, 0, :], start=True, stop=True,
                    skip_group_check=True,
                )

        # ---- delay registers --------------------------------------------
        _, dks = nc.values_load_multi_w_load_instructions(
            idx8[0:1, 0:KTOP].bitcast(I32),
            engines=(ENG.DVE, ENG.Pool, ENG.PE, ENG.Activation),
            min_val=0,
            max_val=L - 1,
            skip_runtime_bounds_check=True,
        )
        # benign reader so the BIR verifier accepts the warm-keeper writes
        # (placed here so the dummies' psum bank frees before the finals)
        nc.scalar.copy(ostg[0:1, 0, 0:1], ps_dum[0:1, 0:1])

        # ---- mix + finals ------------------------------------------------
        def emit_pe_mix(ch, copy_eng):
            base = ch * P
            ps_m = psp.tile([P, D], F32, tag="s", bufs=4, name=f"psm{ch}")
            for k in range(KTOP):
                nc.tensor.matmul(
                    ps_m[:],
                    ident if k == 0 else sid[:, k, :],
                    vt_sb[:, :, bass.ds(dks[k] + base, P)],
                    start=(k == 0),
                    stop=(k == KTOP - 1),
                )
            if copy_eng is nc.scalar:
                nc.scalar.copy(mixs[:, ch, :, :], ps_m[:])
            else:
                copy_eng.tensor_copy(mixs[:, ch, :, :], ps_m[:])

        def emit_dve_mix(ch, ks=range(KTOP)):
            base = ch * P
            mslice = mixs[:, ch, :, :]
            for k in ks:
                if k == 0:
                    nc.vector.tensor_copy(
                        mslice, vt_sb[:, :, bass.ds(dks[0] + base, P)]
                    )
                else:
                    nc.vector.scalar_tensor_tensor(
                        mslice,
                        vt_sb[:, :, bass.ds(dks[k] + base, P)],
                        wbc[:, k : k + 1],
                        mslice,
                        ALU.mult,
                        ALU.add,
                    )

        def emit_actpool_mix(ch, slot, ks=range(KTOP), merge_into=False):
            # ACT: dyn-slice scaled copies into atmp; Pool: accumulate
            base = ch * P
            mslice = mixs[:, ch, :, :]
            first = not merge_into
            for k in ks:
                dst = mslice if first else atmp[:, slot, :, :]
                if first and k == 0:
                    nc.scalar.copy(dst, vt_sb[:, :, bass.ds(dks[0] + base, P)])
                else:
                    nc.scalar.mul(
                        dst, vt_sb[:, :, bass.ds(dks[k] + base, P)], wbc[:, k : k + 1]
                    )
                if not first:
                    nc.gpsimd.tensor_tensor(mslice, atmp[:, slot, :, :], mslice, ALU.add)
                first = False

        def emit_final(ch, slot, copy_eng):
            ps_o = psp.tile(
                [P, D], F32, tag=("a" if slot % 2 == 0 else "b"), bufs=2,
                name=f"pso{ch}",
            )
            for fi in range(FT):
                nc.tensor.matmul(
                    ps_o[:],
                    mixs[:, ch, fi, :],
                    w2p[:, fi, :],
                    start=(fi == 0),
                    stop=(fi == FT - 1),
                )
            stg = ostg[:, slot % 4, :]
            if copy_eng is nc.scalar:
                nc.scalar.mul(stg, ps_o[:], rse_bc[:])
            else:
                copy_eng.tensor_scalar_mul(stg, ps_o[:], rse_bc[:])
            return stg

        # chunk -> engine: PE 0,2,4,6 ; DVE 1,5 ; ACT/Pool 3,7
        emit_pe_mix(0, nc.scalar)
        emit_dve_mix(5)
        emit_actpool_mix(7, 0)
        emit_pe_mix(2, nc.vector)
        emit_pe_mix(4, nc.scalar)
        emit_dve_mix(1)
        emit_actpool_mix(3, 1)
        emit_pe_mix(6, nc.vector)

        # finals in expected mix-readiness order; out DMA spread on queues
        fin_order = [0, 2, 5, 7, 4, 6, 1, 3]
        copy_engs = [nc.vector, nc.scalar, nc.vector, nc.scalar,
                     nc.vector, nc.scalar, nc.vector, nc.scalar]
        dma_engs = [nc.sync] * 8
        for slot, ch in enumerate(fin_order):
            stg = emit_final(ch, slot, copy_engs[slot])
            dma_engs[slot].dma_start(out_d[ch * P : (ch + 1) * P, :], stg)

    return nc


_NC = None
TRACE = False
_LAST_RESULTS = None


def _get_nc():
    global _NC
    if _NC is None:
        _NC = _build()
        _NC.finalize()
    return _NC


def _consts():
    import ml_dtypes

    cst = np.zeros((P, 257), ml_dtypes.bfloat16)
    cst[:, 0:128] = np.eye(P, dtype=np.float32)
    cst[:, 128:257] = 1.0
    return cst


def kernel(queries, keys, values, wq, wk, wv, wo):
    import ml_dtypes

    nc = _get_nc()
    bf = ml_dtypes.bfloat16
    m_b = np.ascontiguousarray(wq @ wk.T, dtype=bf)
    w2_b = np.ascontiguousarray(wv @ wo, dtype=bf)
    cst = _consts()
    in_maps = []
    for c in range(8):
        b, h = divmod(c, 2)
        vrot = np.roll(values[b], -h * HALF, axis=0).T
        vte = np.concatenate([vrot, vrot[:, :HALF]], axis=1)
        in_maps.append(
            {
                "q": np.ascontiguousarray(queries[b], dtype=bf),
                "kt": np.ascontiguousarray(keys[b].T, dtype=bf),
                "vte": np.ascontiguousarray(vte, dtype=bf),
                "m": m_b,
                "w2": w2_b,
                "cst": cst,
            }
        )
    global _LAST_RESULTS
    res = run_bass_kernel_spmd(nc, in_maps, list(range(8)), trace=TRACE)
    _LAST_RESULTS = res
    out = np.empty((B, L, D), np.float32)
    for c in range(8):
        b, h = divmod(c, 2)
        out[b, h * HALF : (h + 1) * HALF] = np.asarray(res.results[c]["out"], np.float32)
    return out


# revision 36
# speedup vs baseline: 1.0177x; 1.0177x over previous
"""AutoCorrelation (Autoformer-style) sparse attention kernel for 8 trn2 cores.

Math (exact refactoring of the reference):
  mean_corr[b,j] = <(sum_i q[b,i]) @ (wq@wk.T), keys[b,j]> / (H*L)
  top7 delays d_k + softmax weights w_k over mean_corr
  out[b,l]      = sum_k w_k * values[b,(l+d_k)%L] @ (wv@wo)

Sharding: core c handles batch b=c//2, output half h=c%2.

v6 schedule (cost-model driven, verified on HW):
  - host-folded weights m = wq@wk.T, w2 = wv@wo; host-extended
    vte = [vrot.T | vrot.T[:, :HALF]] (no on-device wrap copy)
  - 3 DMA queues; a dma_start blocks its issuing engine until the
    transfer ends, so ACT only carries DMAs that finish before its first
    compute, Pool's SWDGE desc-gens run while Pool is otherwise idle,
    and all late bulk rides SP
  - qsum: PE matmuls over early q DMAs + DVE pair-adds for the late
    halves; single transpose hop (tiny [1,128]x[1,1] matmuls into one
    [128,4] psum tile); uT computed directly as 16 single-column
    matmuls (no second hop); s-matmuls lc-major over 5 skewed psum
    banks [512,512,512,384,128] so banks retire in a stream
  - topk without MaxIndex: scores quantized to integers on ACT via the
    f32 +1.5*2^23 trick, packed p = t*4096 + col_idx on Pool (pure
    arithmetic - gpsimd cannot touch PSUM or bitwise ops), per-bank
    top8 via DVE Max, one merge Max; idx/vals recovered arithmetically
  - softmax: max-subtraction makes w0 == 1 so all k0 mix terms are
    plain copies; 1/sum(exp) is broadcast separately into rse_bc and
    applied by the final psum->sbuf copies
  - mix: PE 4 chunks (scaled-identity matmuls) + all 8 finals; DVE 2
    chunks (stt); ACT+Pool 2 chunks (ACT dyn-muls, Pool adds); final
    psums alternate decoupled bank tags; out DMA on SP
"""

import numpy as np
from contextlib import ExitStack

import concourse.bass as bass
import concourse.bacc as bacc
import concourse.mybir as mybir
import concourse.tile as tile
from concourse.bass_utils import run_bass_kernel_spmd

B, L, D, H = 4, 2048, 512, 8
HALF = L // 2
KTOP = 7
EXT = L + HALF
P = 128
FT = D // P
NT = L // P
NCH = HALF // P
F32 = mybir.dt.float32
BF16 = mybir.dt.bfloat16
U32 = mybir.dt.uint32
I32 = mybir.dt.int32
AF = mybir.ActivationFunctionType
ALU = mybir.AluOpType
ENG = mybir.EngineType

QBIAS = 12582912.0  # 1.5*2^23: +QBIAS rounds f32 to integer, ulp stays 1

N_DUMMY = 7        # PE warm-keepers through the topk window
DUMMY_FLOOR = 0.0122  # ms floor for the warm-keepers


def _build():
    nc = bacc.Bacc()
    q_d = nc.dram_tensor("q", [L, D], BF16, kind="ExternalInput")
    kt_d = nc.dram_tensor("kt", [D, L], BF16, kind="ExternalInput")
    vte_d = nc.dram_tensor("vte", [D, EXT], BF16, kind="ExternalInput")
    m_d = nc.dram_tensor("m", [D, D], BF16, kind="ExternalInput")
    w2_d = nc.dram_tensor("w2", [D, D], BF16, kind="ExternalInput")
    cst_d = nc.dram_tensor("cst", [P, 257], BF16, kind="ExternalInput")
    out_d = nc.dram_tensor("out", [HALF, D], BF16, kind="ExternalOutput")

    qdr = q_d.rearrange("(t p) c -> p t c", p=P)
    ktdr = kt_d.rearrange("(f p) l -> p f l", p=P)
    vtdr = vte_d.rearrange("(f p) l -> p f l", p=P)
    mdr = m_d.rearrange("(f p) c -> p f c", p=P)
    w2dr = w2_d.rearrange("(f p) c -> p f c", p=P)

    with tile.TileContext(nc) as tc, ExitStack() as ctx:
        big = ctx.enter_context(tc.tile_pool(name="big", bufs=1))
        sm = ctx.enter_context(tc.tile_pool(name="sm", bufs=1))
        psp = ctx.enter_context(
            tc.tile_pool(name="psp", bufs=1, space=bass.MemorySpace.PSUM)
        )

        cst = sm.tile([P, 257], BF16, tag="cst")
        qp = big.tile([P, NT, D], BF16, tag="qp")
        qh = big.tile([P, 2, D], BF16, tag="qh")
        ktp = big.tile([P, FT, L], BF16, tag="ktp")
        vt_sb = big.tile([P, FT, EXT], BF16, tag="vt")
        mp = big.tile([P, FT, D], BF16, tag="mp")
        w2p = big.tile([P, FT, D], BF16, tag="w2p")
        warm = sm.tile([P, P], BF16, tag="warm")

        # ---- DMA plan (3 queues: SP / ACT / Pool-SWDGE) ------------------
        # SP  : cst | q(0,1) q(2,3) q(4,5) | m01 | kt1 | vte f0 | out x3
        # ACT : q(6,7) q(8,9) | m23 | w2 | vte f1 | vte f3 | out x3
        # Pool: q(12,13) q(14,15) q(10,11) | kt2 kt3 kt0 | vte f2 | out x2
        nc.sync.dma_start(cst[:], cst_d[:])
        ident = cst[:, 0:128]
        ones_col = cst[:, 128:129]
        one_one = cst[0:1, 128:129]
        ones_row = cst[0:1, 129:257]

        nc.sync.dma_start(qp[:, 0:2, :], qdr[:, 0:2, :])
        nc.scalar.dma_start(qp[:, 6:8, :], qdr[:, 6:8, :])
        nc.gpsimd.dma_start(qp[:, 12:14, :], qdr[:, 12:14, :])
        nc.sync.dma_start(qp[:, 2:4, :], qdr[:, 2:4, :])
        nc.scalar.dma_start(qp[:, 8:10, :], qdr[:, 8:10, :])
        nc.gpsimd.dma_start(qp[:, 14:16, :], qdr[:, 14:16, :])
        nc.sync.dma_start(qp[:, 4:6, :], qdr[:, 4:6, :])
        nc.gpsimd.dma_start(qp[:, 10:12, :], qdr[:, 10:12, :])

        nc.sync.dma_start(mp[:, 0:2, :], mdr[:, 0:2, :])
        nc.scalar.dma_start(mp[:, 2:4, :], mdr[:, 2:4, :])
        nc.gpsimd.dma_start(ktp[:, 2:3, :], ktdr[:, 2:3, :])
        nc.sync.dma_start(ktp[:, 1:2, :], ktdr[:, 1:2, :])
        nc.gpsimd.dma_start(ktp[:, 3:4, :], ktdr[:, 3:4, :])
        nc.scalar.dma_start(w2p[:], w2dr[:])
        nc.gpsimd.dma_start(ktp[:, 0:1, 0:HALF], ktdr[:, 0:1, 0:HALF])
        nc.gpsimd.dma_start(ktp[:, 0:1, HALF:L], ktdr[:, 0:1, HALF:L])
        nc.sync.dma_start(vt_sb[:, 0:1, :], vtdr[:, 0:1, :])
        nc.scalar.dma_start(vt_sb[:, 2:3, :], vtdr[:, 2:3, :])

        # index row (exact small integers in f32) on Pool between desc-gens
        civ = sm.tile([1, L], F32, tag="civ")
        nc.gpsimd.iota(
            civ[0:1, 0:L], [[1, L]], base=0, channel_multiplier=0,
            allow_small_or_imprecise_dtypes=True,
        )

        # ---- small tiles -------------------------------------------------
        aux2 = sm.tile([P, 32], BF16, tag="aux2")
        qsT = aux2[:, 0:4]
        uT = aux2[:, 4:8]
        exbf = aux2[0:1, 16:24]
        srow = sm.tile([1, 2 * D], BF16, tag="srow")
        qsum_sb = srow[0:1, 0:D]
        u_sb = srow[0:1, D : 2 * D]
        sfl = sm.tile([1, L], F32, tag="sfl")
        stmp = sm.tile([1, 2 * D], F32, tag="stmp")
        aux3 = sm.tile([1, 64], F32, tag="aux3")
        aux4 = sm.tile([1, 40], F32, tag="aux4")
        vq = aux4[0:1, 0:40]
        vals8 = aux3[0:1, 40:48]
        a8 = aux3[0:1, 48:56]
        t8 = aux3[0:1, 8:16]
        i8f = aux3[0:1, 16:24]
        exf = aux3[0:1, 56:63]
        exin = aux3[0:1, 24:31]
        se = aux3[0:1, 63:64]
        wbc = sm.tile([P, 8], F32, tag="wbc")
        rse = sm.tile([1, 1], F32, tag="rse")
        rse_bc = sm.tile([P, 1], F32, tag="rsebc")
        onesf = sm.tile([1, P], F32, tag="onesf")
        onesf_row = onesf[0:1, :]
        idx8 = sm.tile([1, 8], U32, tag="idx8")
        sid = sm.tile([P, KTOP, P], BF16, tag="sid")
        mixs = big.tile([P, NCH, FT, P], BF16, tag="mixs")
        atmp = sm.tile([P, 8, FT, P], BF16, tag="atmp")
        ostg = sm.tile([P, 4, D], BF16, tag="ostg")

        # ---- PE warm-up: get the p-state ramp going before qsum ----------
        ps_dum = psp.tile([P, D], F32, tag="b", bufs=2)
        nc.vector.memset(warm[:], 0.0)
        nc.vector.memset(onesf[:], 1.0)
        for _ in range(4):
            nc.tensor.matmul(
                ps_dum[:, 0:P], warm[:], warm[:], start=True, stop=True,
                skip_group_check=True,
            )

        # ---- qsum: raw matmuls on early q DMAs + DVE pair-adds on late ---
        nc.vector.tensor_tensor(qh[:, 0, :], qp[:, 8, :], qp[:, 9, :], ALU.add)
        nc.vector.tensor_tensor(qh[:, 1, :], qp[:, 4, :], qp[:, 5, :], ALU.add)

        ps_qsum = psp.tile([1, D], F32, tag="a", bufs=2)
        raw = [12, 13, 0, 1, 14, 15, 6, 7, 2, 3, 10, 11]
        for i, t in enumerate(raw):
            nc.tensor.matmul(
                ps_qsum[:], ones_col, qp[:, t, :], start=(i == 0), stop=False
            )
        for i in range(2):
            nc.tensor.matmul(
                ps_qsum[:], ones_col, qh[:, i, :], start=False, stop=(i == 1)
            )

        def transpose_hop(ps_row, row_sb, colT, hid):
            # half-row copies on DVE, 4 tiny PE transposes into ONE
            # [128,4] psum tile, single col copy out
            nc.vector.tensor_copy(row_sb[0:1, 0 : 2 * P], ps_row[0:1, 0 : 2 * P])
            nc.vector.tensor_copy(row_sb[0:1, 2 * P : 4 * P], ps_row[0:1, 2 * P : 4 * P])
            psT = psp.tile([P, 4], F32, tag="a", bufs=2, name=f"psT{hid}")
            for c in range(FT):
                lo, hi = c * P, (c + 1) * P
                nc.tensor.matmul(
                    psT[:, c : c + 1], row_sb[0:1, lo:hi], one_one,
                    start=True, stop=True, skip_group_check=True,
                )
            nc.vector.tensor_copy(colT[:], psT[:])

        transpose_hop(ps_qsum, qsum_sb, qsT, 0)
        # uT = (qsum @ m)^T computed directly: per output column block,
        # accumulate 4 single-column matmuls -> no second transpose hop
        ps_uT = psp.tile([P, 4], F32, tag="a", bufs=2)
        for dpb in range(FT):
            for i in range(FT):
                nc.tensor.matmul(
                    ps_uT[:, dpb : dpb + 1],
                    mp[:, i, dpb * P : (dpb + 1) * P],
                    qsT[:, i : i + 1],
                    start=(i == 0),
                    stop=(i == FT - 1),
                    skip_group_check=True,
                )
        nc.vector.tensor_copy(uT[:], ps_uT[:])

        # remaining vt pieces ride the idle SP queue
        nc.sync.dma_start(vt_sb[:, 1:2, :], vtdr[:, 1:2, :])
        nc.sync.dma_start(vt_sb[:, 3:4, :], vtdr[:, 3:4, :])

        # ---- scores: 5 skewed banks so the last retires almost with the
        # s-matmul stream end (tiny tail bank -> earlier topk) -------------
        BW = [512, 512, 512, 384, 128]
        BO = [0, 512, 1024, 1536, 1920]
        ps_s = [
            psp.tile([1, BW[i]], F32, tag="s", bufs=4, name=f"ps_s{i}")
            for i in range(5)
        ]
        # s-matmuls interleaved with the pack pipeline per bank so waits
        # attach tightly: ACT quantizes scores to integers (f32 +1.5*2^23
        # trick), Pool packs p = t*4096 + col_idx, DVE runs per-bank maxes
        ford = [2, 3, 1, 0]  # kt piece arrival order
        for lc in range(5):
            for fi, f in enumerate(ford):
                nc.tensor.matmul(
                    ps_s[lc][:],
                    uT[:, f : f + 1],
                    ktp[:, f, BO[lc] : BO[lc] + BW[lc]],
                    start=(fi == 0),
                    stop=(fi == FT - 1),
                )
            stg = stmp[0:1, (lc % 2) * D : (lc % 2) * D + BW[lc]]
            sfs = sfl[0:1, BO[lc] : BO[lc] + BW[lc]]
            nc.scalar.activation(stg, ps_s[lc][:], AF.Copy, bias=QBIAS)
            nc.gpsimd.tensor_scalar(
                sfs, stg, -QBIAS, 4096.0, ALU.add, ALU.mult
            )
            nc.gpsimd.tensor_tensor(
                sfs, sfs, civ[0:1, BO[lc] : BO[lc] + BW[lc]], ALU.add
            )
            nc.vector.max(vq[0:1, 8 * lc : 8 * lc + 8], sfs)
        nc.vector.max(vals8, vq)

        # unpack: t = round(p/4096) via the +QBIAS trick, idx = p - 4096*t
        nc.vector.tensor_scalar(
            a8, vals8, 1.0 / 4096.0, QBIAS, ALU.mult, ALU.add
        )
        nc.vector.tensor_scalar(t8, a8, -QBIAS, None, ALU.add)
        nc.vector.scalar_tensor_tensor(
            i8f, t8, -4096.0, vals8, ALU.mult, ALU.add
        )
        nc.vector.tensor_copy(idx8[0:1, :], i8f)
        nc.vector.tensor_scalar_sub(exin, t8[0:1, 0:KTOP], t8[0:1, 0:1])
        nc.scalar.activation(exf, exin, AF.Exp, scale=1.0 / (H * L))
        # broadcast UNNORMALIZED weights (w0 == 1 exactly, so the k0 mix
        # terms are plain copies); 1/sum(exp) rides a parallel path into
        # rse_bc and is applied by the final psum->sbuf copies
        nc.gpsimd.tensor_copy(exbf[0:1, 0:KTOP], exf)
        nc.vector.tensor_reduce(se, exf, mybir.AxisListType.X, ALU.add)
        ps_w = psp.tile([P, 8], F32, tag="a", bufs=2)
        nc.tensor.matmul(
            ps_w[:, 0:KTOP], ones_row, exbf[0:1, 0:KTOP], start=True, stop=True
        )
        nc.vector.tensor_copy(wbc[:, 0:KTOP], ps_w[:, 0:KTOP])
        for k in range(1, KTOP):
            nc.gpsimd.tensor_scalar_mul(sid[:, k, :], ident, wbc[:, k : k + 1])

        nc.vector.reciprocal(rse[:], se)
        ps_r = psp.tile([P, 1], F32, tag="a", bufs=2)
        nc.tensor.matmul(ps_r[:], onesf_row, rse[:], start=True, stop=True)
        nc.vector.tensor_copy(rse_bc[:], ps_r[:])

        # PE warm-keepers spanning the Max window
        with tc.tile_wait_until(DUMMY_FLOOR):
            for i in range(N_DUMMY):
                nc.tensor.matmul(
                    ps_dum[:], ident, w2p[:, 0, :], start=True, stop=True,
                    skip_group_check=True,
                )

        # ---- delay registers --------------------------------------------
        _, dks = nc.values_load_multi_w_load_instructions(
            idx8[0:1, 0:KTOP].bitcast(I32),
            engines=(ENG.DVE, ENG.Pool, ENG.PE, ENG.Activation),
            min_val=0,
            max_val=L - 1,
            skip_runtime_bounds_check=True,
        )
        # benign reader so the BIR verifier accepts the warm-keeper writes
        # (placed here so the dummies' psum bank frees before the finals)
        nc.scalar.copy(ostg[0:1, 0, 0:1], ps_dum[0:1, 0:1])

        # ---- mix + finals ------------------------------------------------
        def emit_pe_mix(ch, copy_eng):
            base = ch * P
            ps_m = psp.tile([P, D], F32, tag="s", bufs=4, name=f"psm{ch}")
            for k in range(KTOP):
                nc.tensor.matmul(
                    ps_m[:],
                    ident if k == 0 else sid[:, k, :],
                    vt_sb[:, :, bass.ds(dks[k] + base, P)],
                    start=(k == 0),
                    stop=(k == KTOP - 1),
                )
            if copy_eng is nc.scalar:
                nc.scalar.copy(mixs[:, ch, :, :], ps_m[:])
            else:
                copy_eng.tensor_copy(mixs[:, ch, :, :], ps_m[:])

        def emit_dve_mix(ch, ks=range(KTOP), pool_tail=0, slots=()):
            base = ch * P
            mslice = mixs[:, ch, :, :]
            ks = list(ks)
            tail = ks[len(ks) - pool_tail :] if pool_tail else []
            for i, k in enumerate(tail):
                a = atmp[:, slots[i], :, :]
                nc.gpsimd.tensor_copy(a, vt_sb[:, :, bass.ds(dks[k] + base, P)])
                nc.gpsimd.tensor_scalar_mul(a, a, wbc[:, k : k + 1])
            for k in ks[: len(ks) - pool_tail]:
                if k == 0:
                    nc.vector.tensor_copy(
                        mslice, vt_sb[:, :, bass.ds(dks[0] + base, P)]
                    )
                else:
                    nc.vector.scalar_tensor_tensor(
                        mslice,
                        vt_sb[:, :, bass.ds(dks[k] + base, P)],
                        wbc[:, k : k + 1],
                        mslice,
                        ALU.mult,
                        ALU.add,
                    )
            for i in range(pool_tail):
                nc.vector.tensor_tensor(
                    mslice, atmp[:, slots[i], :, :], mslice, ALU.add
                )

        def emit_actpool_mix(ch, slot, ks=range(KTOP), pool_tail=0, slots=()):
            # ACT: dyn-slice scaled copies into atmp; Pool: accumulate;
            # optional tail terms built entirely on Pool
            base = ch * P
            mslice = mixs[:, ch, :, :]
            ks = list(ks)
            tail = ks[len(ks) - pool_tail :] if pool_tail else []
            for i, k in enumerate(tail):
                a = atmp[:, slots[i], :, :]
                nc.gpsimd.tensor_copy(a, vt_sb[:, :, bass.ds(dks[k] + base, P)])
                nc.gpsimd.tensor_scalar_mul(a, a, wbc[:, k : k + 1])
            first = True
            for k in ks[: len(ks) - pool_tail]:
                dst = mslice if first else atmp[:, slot, :, :]
                if first and k == 0:
                    nc.scalar.copy(dst, vt_sb[:, :, bass.ds(dks[0] + base, P)])
                else:
                    nc.scalar.mul(
                        dst, vt_sb[:, :, bass.ds(dks[k] + base, P)], wbc[:, k : k + 1]
                    )
                if not first:
                    nc.gpsimd.tensor_tensor(mslice, atmp[:, slot, :, :], mslice, ALU.add)
                first = False
            for i in range(pool_tail):
                nc.gpsimd.tensor_tensor(
                    mslice, atmp[:, slots[i], :, :], mslice, ALU.add
                )

        def emit_final(ch, slot, copy_eng):
            ps_o = psp.tile(
                [P, D], F32, tag=("a" if slot % 2 == 0 else "b"), bufs=2,
                name=f"pso{ch}",
            )
            for fi in range(FT):
                nc.tensor.matmul(
                    ps_o[:],
                    mixs[:, ch, fi, :],
                    w2p[:, fi, :],
                    start=(fi == 0),
                    stop=(fi == FT - 1),
                )
            stg = ostg[:, slot % 4, :]
            if copy_eng is nc.scalar:
                nc.scalar.mul(stg, ps_o[:], rse_bc[:])
            else:
                copy_eng.tensor_scalar_mul(stg, ps_o[:], rse_bc[:])
            return stg

        # chunk -> engine: PE 0,2,4,6 ; DVE 1,5 ; ACT/Pool 3,7
        emit_pe_mix(0, nc.scalar)
        emit_dve_mix(5)
        emit_actpool_mix(7, 0)
        emit_pe_mix(2, nc.vector)
        emit_pe_mix(4, nc.scalar)
        emit_dve_mix(1, pool_tail=3, slots=(2, 3, 4))
        emit_actpool_mix(3, 1, pool_tail=2, slots=(5, 6))
        emit_pe_mix(6, nc.vector)

        # finals in expected mix-readiness order; out DMA spread on queues
        fin_order = [0, 2, 5, 7, 4, 6, 1, 3]
        copy_engs = [nc.vector, nc.scalar, nc.vector, nc.scalar,
                     nc.vector, nc.scalar, nc.vector, nc.scalar]
        dma_engs = [nc.sync] * 8
        for slot, ch in enumerate(fin_order):
            stg = emit_final(ch, slot, copy_engs[slot])
            dma_engs[slot].dma_start(out_d[ch * P : (ch + 1) * P, :], stg)

    return nc


_NC = None
TRACE = False
_LAST_RESULTS = None


def _get_nc():
    global _NC
    if _NC is None:
        _NC = _build()
        _NC.finalize()
    return _NC


def _consts():
    import ml_dtypes

    cst = np.zeros((P, 257), ml_dtypes.bfloat16)
    cst[:, 0:128] = np.eye(P, dtype=np.float32)
    cst[:, 128:257] = 1.0
    return cst


def kernel(queries, keys, values, wq, wk, wv, wo):
    import ml_dtypes

    nc = _get_nc()
    bf = ml_dtypes.bfloat16
    m_b = np.ascontiguousarray(wq @ wk.T, dtype=bf)
    w2_b = np.ascontiguousarray(wv @ wo, dtype=bf)
    cst = _consts()
    in_maps = []
    for c in range(8):
        b, h = divmod(c, 2)
        vrot = np.roll(values[b], -h * HALF, axis=0).T
        vte = np.concatenate([vrot, vrot[:, :HALF]], axis=1)
        in_maps.append(
            {
                "q": np.ascontiguousarray(queries[b], dtype=bf),
                "kt": np.ascontiguousarray(keys[b].T, dtype=bf),
                "vte": np.ascontiguousarray(vte, dtype=bf),
                "m": m_b,
                "w2": w2_b,
                "cst": cst,
            }
        )
    global _LAST_RESULTS
    res = run_bass_kernel_spmd(nc, in_maps, list(range(8)), trace=TRACE)
    _LAST_RESULTS = res
    out = np.empty((B, L, D), np.float32)
    for c in range(8):
        b, h = divmod(c, 2)
        out[b, h * HALF : (h + 1) * HALF] = np.asarray(res.results[c]["out"], np.float32)
    return out
